# revision 4
# baseline (speedup 1.0000x reference)
"""DMPNNConv kernel for 8 Trainium2 NeuronCores (v2).

  h_n = relu([x ; h_e] @ W_i_w.T + W_i_b)          [N, D]
  m   = einsum('kn,nd->d', bond_n, h_n)            [D]
  h   = relu(h_n + m @ W_m_w.T + W_m_b)            [N, D]

Sharding: N (edge dim) split 8 ways; weights replicated; single [D]
all-reduce of the message m between the two passes.

v2 design (vs v1 two-pass-over-DRAM baseline):
  * h_n stays RESIDENT IN SBUF (bf16, [128, 63488] = 124 KB/partition)
    -- no 32 MB DRAM scratch round-trip.
  * Token permutation token = 2048*s + 16*p + j gives every DMA
    8 KB contiguous per partition (1 MB per DMA instruction) instead
    of 512 B packets; bond stays in natural order and is index-
    permuted on chip via a strided AP view fed to the PE.
  * pass 2 adds the broadcast message c via a rank-1 PE accumulate
    (ones^T @ c_row) into the transpose PSUM tile; DVE tensor_scalar
    max(x,0) drains PSUM straight to the f32 output tile.
"""

import os
import sys

sys.path.insert(0, "/opt/trn_rl_repo")

import numpy as np

N, D, K = 500000, 128, 32
CORES = 8
SUP = 2048                   # tokens per super-tile (one DMA)
JC = SUP // 128              # 16 rows per partition per super-tile
NSUP = 31                    # super-tiles per core
N_SH = NSUP * SUP            # 63488 padded rows per core
NT = NSUP * 4                # 124 compute tiles (512 tokens each)
N_PAD = CORES * N_SH         # 507904

_cache = {}
last_results = None


def _build():
    import concourse.bass as bass
    import concourse.bacc as bacc
    import concourse.tile as tile
    import concourse.mybir as mybir
    from concourse import masks

    f32 = mybir.dt.float32
    bf16 = mybir.dt.bfloat16
    AF = mybir.ActivationFunctionType
    ALU = mybir.AluOpType

    nc = bacc.Bacc("TRN2", target_bir_lowering=False, debug=False,
                   num_devices=CORES)

    x_d = nc.dram_tensor("x", [N_SH, D], f32, kind="ExternalInput").ap()
    he_d = nc.dram_tensor("h_e", [N_SH, D], f32, kind="ExternalInput").ap()
    bond_d = nc.dram_tensor("bond_n", [K, N_SH], f32, kind="ExternalInput").ap()
    wi_d = nc.dram_tensor("W_i_w", [D, 2 * D], f32, kind="ExternalInput").ap()
    bi_d = nc.dram_tensor("W_i_b", [D], f32, kind="ExternalInput").ap()
    wm_d = nc.dram_tensor("W_m_w", [D, D], f32, kind="ExternalInput").ap()
    bm_d = nc.dram_tensor("W_m_b", [D], f32, kind="ExternalInput").ap()
    h_d = nc.dram_tensor("h", [N_SH, D], f32, kind="ExternalOutput").ap()

    # token(s, p, j) = 2048*s + 16*p + j  ->  8 KB contiguous per partition
    x_re = x_d.rearrange("(s p j) d -> s p j d", p=128, j=JC)
    he_re = he_d.rearrange("(s p j) d -> s p j d", p=128, j=JC)
    h_re = h_d.rearrange("(s p j) d -> s p j d", p=128, j=JC)
    bond_re = bond_d.rearrange("k (s u) -> s k u", u=SUP)

    with tile.TileContext(nc) as tc:
        import contextlib
        ctx = contextlib.ExitStack()
        with ctx:
            pers = ctx.enter_context(tc.tile_pool(name="pers", bufs=1))
            iosup = ctx.enter_context(tc.tile_pool(name="iosup", bufs=3))
            iosm = ctx.enter_context(tc.tile_pool(name="iosm", bufs=6))
            ioout = ctx.enter_context(tc.tile_pool(name="ioout", bufs=2))
            ps_t = ctx.enter_context(tc.tile_pool(name="ps_t", bufs=4, space="PSUM"))
            ps_z = ctx.enter_context(tc.tile_pool(name="ps_z", bufs=2, space="PSUM"))
            ps_w = ctx.enter_context(tc.tile_pool(name="ps_w", bufs=2, space="PSUM"))
            dram = ctx.enter_context(tc.tile_pool(name="dram", bufs=1, space="DRAM"))

            # ---- one-time setup -------------------------------------------
            ident_bf = pers.tile([128, 128], bf16)
            masks.make_identity(nc, ident_bf[:])
            ident_f = pers.tile([128, 128], f32)
            masks.make_identity(nc, ident_f[:])

            ones32 = pers.tile([K, 128], bf16)
            nc.gpsimd.memset(ones32[:], 1.0)
            ones1 = pers.tile([1, 128], f32)
            nc.gpsimd.memset(ones1[:], 1.0)

            b1_col = pers.tile([128, 1], f32)
            nc.sync.dma_start(b1_col[:, 0], bi_d[:])
            b2_col = pers.tile([128, 1], f32)
            nc.sync.dma_start(b2_col[:, 0], bm_d[:])

            # W_i_w [D, 2D] -> bf16 -> transpose halves -> W1aT/W1bT [j, d]
            wi_sb = pers.tile([128, 2 * D], f32)
            nc.sync.dma_start(wi_sb[:], wi_d[:])
            wi_bf = pers.tile([128, 2 * D], bf16)
            nc.vector.tensor_copy(wi_bf[:], wi_sb[:])
            w1t = pers.tile([128, 2 * D], bf16)  # [j, (half d)]
            for half in range(2):
                tp = ps_t.tile([128, 128], bf16, tag="tr")
                nc.tensor.transpose(tp[:], wi_bf[:, 128 * half:128 * (half + 1)],
                                    ident_bf[:])
                nc.vector.tensor_copy(w1t[:, 128 * half:128 * (half + 1)], tp[:])

            # W_m_w [D, D] -> WmT [d, d'] f32 (precision-critical path)
            wm_sb = pers.tile([128, D], f32)
            nc.sync.dma_start(wm_sb[:], wm_d[:])
            wmt = pers.tile([128, D], f32)
            tpm = ps_z.tile([128, 128], f32, tag="z")
            nc.tensor.transpose(tpm[:], wm_sb[:], ident_f[:])
            nc.vector.tensor_copy(wmt[:], tpm[:])

            m_parts = pers.tile([128, NT], f32)
            hn_all = pers.tile([128, N_SH], bf16)   # SBUF-resident h_n.T
            m_in = dram.tile([128], f32)
            m_out = dram.tile([128], f32, addr_space="Shared")

            # ---- pass 1 ----------------------------------------------------
            for s in range(NSUP):
                x_bf = iosup.tile([128, SUP], bf16)
                nc.gpsimd.dma_start(
                    x_bf[:].rearrange("p (j d) -> p j d", j=JC), x_re[s])
                he_bf = iosup.tile([128, SUP], bf16)
                nc.gpsimd.dma_start(
                    he_bf[:].rearrange("p (j d) -> p j d", j=JC), he_re[s])
                b_bf = iosup.tile([K, SUP], bf16)
                nc.gpsimd.dma_start(b_bf[:], bond_re[s])
                # b_view[k, g, jj, p] = b_bf[k, 16p + 4g + jj]
                b_view = b_bf[:].rearrange("k (p a b) -> k a b p",
                                           p=128, a=4, b=4)

                for g in range(4):
                    xt_ps = ps_t.tile([128, 512], f32, tag="tr")
                    het_ps = ps_t.tile([128, 512], f32, tag="tr")
                    for jj in range(4):
                        bsl = slice(128 * (4 * g + jj), 128 * (4 * g + jj + 1))
                        psl = slice(128 * jj, 128 * (jj + 1))
                        nc.tensor.matmul(xt_ps[:, psl], x_bf[:, bsl],
                                         ident_bf[:], start=True, stop=True,
                                         skip_group_check=True)
                        nc.tensor.matmul(het_ps[:, psl], he_bf[:, bsl],
                                         ident_bf[:], start=True, stop=True,
                                         skip_group_check=True)
                    xt_bf = iosm.tile([128, 512], bf16)
                    nc.vector.tensor_copy(xt_bf[:], xt_ps[:])
                    het_bf = iosm.tile([128, 512], bf16)
                    nc.vector.tensor_copy(het_bf[:], het_ps[:])

                    z_ps = ps_z.tile([128, 512], f32, tag="z")
                    nc.tensor.matmul(z_ps[:], w1t[:, 0:128], xt_bf[:],
                                     start=True, stop=False)
                    nc.tensor.matmul(z_ps[:], w1t[:, 128:256], het_bf[:],
                                     start=False, stop=True)

                    wb_ps = ps_w.tile([128, 512], f32, tag="wb")
                    nc.tensor.matmul(wb_ps[:], ones32[:], b_view[:, g],
                                     start=True, stop=True)

                    ti = 4 * s + g
                    hsl = slice(512 * ti, 512 * (ti + 1))
                    nc.scalar.activation(hn_all[:, hsl], z_ps[:], AF.Relu,
                                         bias=b1_col[:])
                    junk = iosm.tile([128, 512], bf16)
                    nc.vector.scalar_tensor_tensor(
                        junk[:], hn_all[:, hsl], 1.0, wb_ps[:],
                        ALU.mult, ALU.mult,
                        accum_out=m_parts[:, ti:ti + 1])

            # ---- m all-reduce + c ----------------------------------------
            m_col = pers.tile([128, 1], f32)
            nc.vector.reduce_sum(m_col[:], m_parts[:], axis=mybir.AxisListType.X)
            nc.sync.dma_start(m_in[:], m_col[:, 0])
            nc.gpsimd.collective_compute(
                "AllReduce", ALU.add,
                replica_groups=[list(range(CORES))],
                ins=[m_in[:].opt()], outs=[m_out[:].opt()])
            m_sb = pers.tile([128, 1], f32)
            nc.sync.dma_start(m_sb[:, 0], m_out[:])

            c_ps = ps_z.tile([128, 1], f32, tag="z")
            nc.tensor.matmul(c_ps[:], wmt[:], m_sb[:], start=True, stop=True)
            c_col = pers.tile([128, 1], f32)
            nc.vector.tensor_tensor(c_col[:], c_ps[:], b2_col[:], ALU.add)

            c_rps = ps_w.tile([1, 128], f32, tag="wb")
            nc.tensor.transpose(c_rps[:], c_col[:], ident_f[:])
            c_row4 = pers.tile([1, 512], f32)
            for r in range(4):
                nc.vector.tensor_copy(c_row4[:, 128 * r:128 * (r + 1)],
                                      c_rps[:])

            # ---- pass 2 ----------------------------------------------------
            for s in range(NSUP):
                ho = ioout.tile([128, SUP], f32)
                for g in range(4):
                    ti = 4 * s + g
                    ht_ps = ps_t.tile([128, 512], f32, tag="tr")
                    # broadcast c along free dim first (rank-1: ones^T @
                    # c_row4, start=True clears the bank), then accumulate
                    # the 4 transposed h_n blocks onto it.
                    nc.tensor.matmul(ht_ps[:], ones1[:], c_row4[:],
                                     start=True, stop=False,
                                     skip_group_check=True)
                    for jj in range(4):
                        csl = slice(512 * ti + 128 * jj,
                                    512 * ti + 128 * (jj + 1))
                        psl = slice(128 * jj, 128 * (jj + 1))
                        nc.tensor.matmul(ht_ps[:, psl], hn_all[:, csl],
                                         ident_bf[:], start=False,
                                         stop=(jj == 3),
                                         skip_group_check=True)
                    nc.vector.tensor_scalar_max(
                        ho[:, 512 * g:512 * (g + 1)], ht_ps[:], 0.0)
                nc.scalar.dma_start(h_re[s],
                                    ho[:].rearrange("p (j d) -> p j d", j=JC))

    nc.compile()
    return nc


def _get_nc():
    if "nc" not in _cache:
        _cache["nc"] = _build()
    return _cache["nc"]


def _ensure_ntff_hook():
    """Register the axon NTFF profile hook if the image's antenv lacks it."""
    import types
    try:
        import antenv.axon_hooks  # noqa: F401
        return
    except ImportError:
        pass
    try:
        import antenv
        from trn_agent_boot.trn_boot import _ntff_profile_via_ctypes
        mod = types.ModuleType("antenv.axon_hooks")
        _h = {"hook": None}
        mod.set_axon_ntff_profile_hook = lambda h: _h.__setitem__("hook", h)
        mod.get_axon_ntff_profile_hook = lambda: _h["hook"]
        sys.modules["antenv.axon_hooks"] = mod
        antenv.axon_hooks = mod
        hook = _ntff_profile_via_ctypes("/opt/axon/libaxon_pjrt.so")
        if hook is not None:
            mod.set_axon_ntff_profile_hook(hook)
    except Exception:
        pass


def kernel(**inputs):
    global last_results
    from concourse.bass_utils import run_bass_kernel_spmd

    x = np.ascontiguousarray(np.asarray(inputs["x"], dtype=np.float32))
    he = np.ascontiguousarray(np.asarray(inputs["h_e"], dtype=np.float32))
    bond = np.ascontiguousarray(np.asarray(inputs["bond_n"], dtype=np.float32))
    wi = np.ascontiguousarray(np.asarray(inputs["W_i_w"], dtype=np.float32))
    bi = np.ascontiguousarray(np.asarray(inputs["W_i_b"], dtype=np.float32))
    wm = np.ascontiguousarray(np.asarray(inputs["W_m_w"], dtype=np.float32))
    bm = np.ascontiguousarray(np.asarray(inputs["W_m_b"], dtype=np.float32))

    n = x.shape[0]
    pad = N_PAD - n
    xp = np.concatenate([x, np.zeros((pad, D), np.float32)], 0)
    hep = np.concatenate([he, np.zeros((pad, D), np.float32)], 0)
    bondp = np.concatenate([bond, np.zeros((K, pad), np.float32)], 1)

    in_maps = []
    for c in range(CORES):
        sl = slice(c * N_SH, (c + 1) * N_SH)
        in_maps.append({
            "x": xp[sl],
            "h_e": hep[sl],
            "bond_n": np.ascontiguousarray(bondp[:, sl]),
            "W_i_w": wi, "W_i_b": bi, "W_m_w": wm, "W_m_b": bm,
        })

    nc = _get_nc()
    trace = os.environ.get("BASS_KERNEL_TRACE", "0") == "1"
    if trace:
        _ensure_ntff_hook()
    res = run_bass_kernel_spmd(nc, in_maps, core_ids=list(range(CORES)),
                               trace=trace)
    last_results = res
    out = np.concatenate([r["h"] for r in res.results], 0)[:n]
    return np.ascontiguousarray(out)


# revision 5
# speedup vs baseline: 1.2118x; 1.2118x over previous
"""DMPNNConv kernel for 8 Trainium2 NeuronCores (v3).

  h_n = relu([x ; h_e] @ W_i_w.T + W_i_b)          [N, D]
  m   = einsum('kn,nd->d', bond_n, h_n)            [D]
  h   = relu(h_n + m @ W_m_w.T + W_m_b)            [N, D]

Sharding: N (edge dim) split 8 ways; weights replicated; single [D]
all-reduce of the message m between the two passes.

v3 design (the PE in this environment is pinned at the throttled
1.2 GHz clock, so PE cycles are the scarce resource):
  * x / h_e / bond are cast to bf16 on the HOST (same rounding the
    previous on-chip cast-DMA applied) -> HBM read traffic halved;
    the output h is written bf16 and upcast on the host (max elementwise
    error 2^-8 = 0.39% << the 2e-2 gate).
  * x.T / h_e.T are loaded straight from DRAM with the xbar
    DMA-transpose (2-byte dtype, [2048,128] -> [128,2048]) -- ZERO
    PE transpose work and zero DVE drains in pass 1.
  * h_n stays resident in SBUF ([128, 63488] bf16, 124 KB/partition).
  * The matmul moving operands use stride-16 column views so h_n is
    produced in the permuted order token = 2048 s + 16 p + j; pass 2's
    transposed output tiles then give 4 KB-contiguous DMA writes.
  * pass 2: +c is an ACT *bias* in the [d, token] layout (c is
    per-partition there) fused with relu; PE only does the 128x128
    transposes back; DVE drains PSUM into the bf16 output tile.
"""

import os
import sys

sys.path.insert(0, "/opt/trn_rl_repo")

import numpy as np
import ml_dtypes

N, D, K = 500000, 128, 32
CORES = 8
SUP = 2048                   # tokens per super-tile (one DMA)
JC = SUP // 128              # 16 rows per partition per super-tile
NSUP = 31                    # super-tiles per core
N_SH = NSUP * SUP            # 63488 padded rows per core
NT = NSUP * 4                # 124 compute tiles (512 tokens each)
N_PAD = CORES * N_SH         # 507904

_cache = {}
last_results = None


def _build():
    import concourse.bass as bass
    import concourse.bacc as bacc
    import concourse.tile as tile
    import concourse.mybir as mybir
    from concourse import masks

    f32 = mybir.dt.float32
    bf16 = mybir.dt.bfloat16
    AF = mybir.ActivationFunctionType
    ALU = mybir.AluOpType

    nc = bacc.Bacc("TRN2", target_bir_lowering=False, debug=False,
                   num_devices=CORES)

    x_d = nc.dram_tensor("x", [N_SH, D], bf16, kind="ExternalInput").ap()
    he_d = nc.dram_tensor("h_e", [N_SH, D], bf16, kind="ExternalInput").ap()
    bond_d = nc.dram_tensor("bond_n", [K, N_SH], bf16, kind="ExternalInput").ap()
    wi_d = nc.dram_tensor("W_i_w", [D, 2 * D], f32, kind="ExternalInput").ap()
    bi_d = nc.dram_tensor("W_i_b", [D], f32, kind="ExternalInput").ap()
    wm_d = nc.dram_tensor("W_m_w", [D, D], f32, kind="ExternalInput").ap()
    bm_d = nc.dram_tensor("W_m_b", [D], f32, kind="ExternalInput").ap()
    h_d = nc.dram_tensor("h", [N_SH, D], bf16, kind="ExternalOutput").ap()

    # output rows 2048*s + 16*p + j -> 4 KB contiguous per partition
    h_re = h_d.rearrange("(s p j) d -> s p j d", p=128, j=JC)
    bond_re = bond_d.rearrange("k (s u) -> s k u", u=SUP)

    with tile.TileContext(nc) as tc:
        import contextlib
        ctx = contextlib.ExitStack()
        with ctx:
            pers = ctx.enter_context(tc.tile_pool(name="pers", bufs=1))
            iosup = ctx.enter_context(tc.tile_pool(name="iosup", bufs=3))
            iosm = ctx.enter_context(tc.tile_pool(name="iosm", bufs=6))
            ioout = ctx.enter_context(tc.tile_pool(name="ioout", bufs=2))
            ps_t = ctx.enter_context(tc.tile_pool(name="ps_t", bufs=4, space="PSUM"))
            ps_z = ctx.enter_context(tc.tile_pool(name="ps_z", bufs=2, space="PSUM"))
            ps_w = ctx.enter_context(tc.tile_pool(name="ps_w", bufs=2, space="PSUM"))
            dram = ctx.enter_context(tc.tile_pool(name="dram", bufs=1, space="DRAM"))

            # ---- one-time setup -------------------------------------------
            ident_bf = pers.tile([128, 128], bf16)
            masks.make_identity(nc, ident_bf[:])
            ident_f = pers.tile([128, 128], f32)
            masks.make_identity(nc, ident_f[:])

            ones32 = pers.tile([K, 128], bf16)
            nc.gpsimd.memset(ones32[:], 1.0)

            b1_col = pers.tile([128, 1], f32)
            nc.sync.dma_start(b1_col[:, 0], bi_d[:])
            b2_col = pers.tile([128, 1], f32)
            nc.sync.dma_start(b2_col[:, 0], bm_d[:])

            # W_i_w [D, 2D] -> bf16 -> transpose halves -> W1aT/W1bT [j, d]
            wi_sb = pers.tile([128, 2 * D], f32)
            nc.sync.dma_start(wi_sb[:], wi_d[:])
            wi_bf = pers.tile([128, 2 * D], bf16)
            nc.vector.tensor_copy(wi_bf[:], wi_sb[:])
            w1t = pers.tile([128, 2 * D], bf16)  # [j, (half d)]
            for half in range(2):
                tp = ps_t.tile([128, 128], bf16, tag="tr")
                nc.tensor.transpose(tp[:], wi_bf[:, 128 * half:128 * (half + 1)],
                                    ident_bf[:])
                nc.vector.tensor_copy(w1t[:, 128 * half:128 * (half + 1)], tp[:])

            # W_m_w [D, D] -> WmT [d, d'] f32 (precision-critical path)
            wm_sb = pers.tile([128, D], f32)
            nc.sync.dma_start(wm_sb[:], wm_d[:])
            wmt = pers.tile([128, D], f32)
            tpm = ps_z.tile([128, 128], f32, tag="z")
            nc.tensor.transpose(tpm[:], wm_sb[:], ident_f[:])
            nc.vector.tensor_copy(wmt[:], tpm[:])

            m_parts = pers.tile([128, NT], f32)
            hn_all = pers.tile([128, N_SH], bf16)   # SBUF-resident h_n.T
            m_in = dram.tile([128], f32)
            m_out = dram.tile([128], f32, addr_space="Shared")

            # ---- pass 1 ----------------------------------------------------
            for s in range(NSUP):
                xt = iosup.tile([128, SUP], bf16)
                nc.sync.dma_start_transpose(
                    xt[:], x_d[SUP * s:SUP * (s + 1), :])
                het = iosup.tile([128, SUP], bf16)
                nc.scalar.dma_start_transpose(
                    het[:], he_d[SUP * s:SUP * (s + 1), :])
                b_bf = iosup.tile([K, SUP], bf16)
                nc.gpsimd.dma_start(b_bf[:], bond_re[s])

                # strided column views: [*, g, jj, p] = col 16p + 4g + jj
                xt_v = xt[:].rearrange("d (p a b) -> d a b p", p=128, a=4, b=4)
                het_v = het[:].rearrange("d (p a b) -> d a b p", p=128, a=4, b=4)
                b_v = b_bf[:].rearrange("k (p a b) -> k a b p", p=128, a=4, b=4)

                for g in range(4):
                    z_ps = ps_z.tile([128, 512], f32, tag="z")
                    nc.tensor.matmul(z_ps[:], w1t[:, 0:128], xt_v[:, g],
                                     start=True, stop=False)
                    nc.tensor.matmul(z_ps[:], w1t[:, 128:256], het_v[:, g],
                                     start=False, stop=True)

                    wb_ps = ps_w.tile([128, 512], f32, tag="wb")
                    nc.tensor.matmul(wb_ps[:], ones32[:], b_v[:, g],
                                     start=True, stop=True)

                    ti = 4 * s + g
                    hsl = slice(512 * ti, 512 * (ti + 1))
                    nc.scalar.activation(hn_all[:, hsl], z_ps[:], AF.Relu,
                                         bias=b1_col[:])
                    junk = iosm.tile([128, 512], bf16)
                    nc.vector.scalar_tensor_tensor(
                        junk[:], hn_all[:, hsl], 1.0, wb_ps[:],
                        ALU.mult, ALU.mult,
                        accum_out=m_parts[:, ti:ti + 1])

            # ---- m all-reduce + c ----------------------------------------
            m_col = pers.tile([128, 1], f32)
            nc.vector.reduce_sum(m_col[:], m_parts[:], axis=mybir.AxisListType.X)
            nc.sync.dma_start(m_in[:], m_col[:, 0])
            nc.gpsimd.collective_compute(
                "AllReduce", ALU.add,
                replica_groups=[list(range(CORES))],
                ins=[m_in[:].opt()], outs=[m_out[:].opt()])
            m_sb = pers.tile([128, 1], f32)
            nc.sync.dma_start(m_sb[:, 0], m_out[:])

            c_ps = ps_z.tile([128, 1], f32, tag="z")
            nc.tensor.matmul(c_ps[:], wmt[:], m_sb[:], start=True, stop=True)
            c_col = pers.tile([128, 1], f32)
            nc.vector.tensor_tensor(c_col[:], c_ps[:], b2_col[:], ALU.add)

            # ---- pass 2 ----------------------------------------------------
            for s in range(NSUP):
                h_sup = ioout.tile([128, SUP], bf16)
                for g in range(4):
                    ti = 4 * s + g
                    t_bf = iosm.tile([128, 512], bf16)
                    nc.scalar.activation(t_bf[:],
                                         hn_all[:, 512 * ti:512 * (ti + 1)],
                                         AF.Relu, bias=c_col[:])
                    ht_ps = ps_t.tile([128, 512], f32, tag="tr")
                    for jj in range(4):
                        psl = slice(128 * jj, 128 * (jj + 1))
                        nc.tensor.matmul(ht_ps[:, psl], t_bf[:, psl],
                                         ident_bf[:], start=True, stop=True,
                                         skip_group_check=True)
                    nc.vector.tensor_copy(h_sup[:, 512 * g:512 * (g + 1)],
                                          ht_ps[:])
                nc.scalar.dma_start(h_re[s],
                                    h_sup[:].rearrange("p (j d) -> p j d",
                                                       j=JC))

    nc.compile()
    return nc


def _get_nc():
    if "nc" not in _cache:
        _cache["nc"] = _build()
    return _cache["nc"]


def _ensure_ntff_hook():
    """Register the axon NTFF profile hook if the image's antenv lacks it."""
    import types
    try:
        import antenv.axon_hooks  # noqa: F401
        return
    except ImportError:
        pass
    try:
        import antenv
        from trn_agent_boot.trn_boot import _ntff_profile_via_ctypes
        mod = types.ModuleType("antenv.axon_hooks")
        _h = {"hook": None}
        mod.set_axon_ntff_profile_hook = lambda h: _h.__setitem__("hook", h)
        mod.get_axon_ntff_profile_hook = lambda: _h["hook"]
        sys.modules["antenv.axon_hooks"] = mod
        antenv.axon_hooks = mod
        hook = _ntff_profile_via_ctypes("/opt/axon/libaxon_pjrt.so")
        if hook is not None:
            mod.set_axon_ntff_profile_hook(hook)
    except Exception:
        pass


def kernel(**inputs):
    global last_results
    from concourse.bass_utils import run_bass_kernel_spmd

    bf = ml_dtypes.bfloat16
    x = np.asarray(inputs["x"], dtype=np.float32).astype(bf)
    he = np.asarray(inputs["h_e"], dtype=np.float32).astype(bf)
    bond = np.asarray(inputs["bond_n"], dtype=np.float32).astype(bf)
    wi = np.ascontiguousarray(np.asarray(inputs["W_i_w"], dtype=np.float32))
    bi = np.ascontiguousarray(np.asarray(inputs["W_i_b"], dtype=np.float32))
    wm = np.ascontiguousarray(np.asarray(inputs["W_m_w"], dtype=np.float32))
    bm = np.ascontiguousarray(np.asarray(inputs["W_m_b"], dtype=np.float32))

    n = x.shape[0]
    pad = N_PAD - n
    xp = np.concatenate([x, np.zeros((pad, D), bf)], 0)
    hep = np.concatenate([he, np.zeros((pad, D), bf)], 0)
    bondp = np.concatenate([bond, np.zeros((K, pad), bf)], 1)

    in_maps = []
    for c in range(CORES):
        sl = slice(c * N_SH, (c + 1) * N_SH)
        in_maps.append({
            "x": np.ascontiguousarray(xp[sl]),
            "h_e": np.ascontiguousarray(hep[sl]),
            "bond_n": np.ascontiguousarray(bondp[:, sl]),
            "W_i_w": wi, "W_i_b": bi, "W_m_w": wm, "W_m_b": bm,
        })

    nc = _get_nc()
    trace = os.environ.get("BASS_KERNEL_TRACE", "0") == "1"
    if trace:
        _ensure_ntff_hook()
    res = run_bass_kernel_spmd(nc, in_maps, core_ids=list(range(CORES)),
                               trace=trace)
    last_results = res
    out = np.concatenate([np.asarray(r["h"]) for r in res.results], 0)[:n]
    return np.ascontiguousarray(out.astype(np.float32))


# revision 7
# speedup vs baseline: 1.4326x; 1.1822x over previous
"""DMPNNConv kernel for 8 Trainium2 NeuronCores (v3).

  h_n = relu([x ; h_e] @ W_i_w.T + W_i_b)          [N, D]
  m   = einsum('kn,nd->d', bond_n, h_n)            [D]
  h   = relu(h_n + m @ W_m_w.T + W_m_b)            [N, D]

Sharding: N (edge dim) split 8 ways; weights replicated; single [D]
all-reduce of the message m between the two passes.

v3 design (the PE in this environment is pinned at the throttled
1.2 GHz clock, so PE cycles are the scarce resource):
  * x / h_e / bond are cast to bf16 on the HOST (same rounding the
    previous on-chip cast-DMA applied) -> HBM read traffic halved;
    the output h is written bf16 and upcast on the host (max elementwise
    error 2^-8 = 0.39% << the 2e-2 gate).
  * x.T / h_e.T are loaded straight from DRAM with the xbar
    DMA-transpose (2-byte dtype, [2048,128] -> [128,2048]) -- ZERO
    PE transpose work and zero DVE drains in pass 1.
  * h_n stays resident in SBUF ([128, 63488] bf16, 124 KB/partition).
  * The matmul moving operands use stride-16 column views so h_n is
    produced in the permuted order token = 2048 s + 16 p + j; pass 2's
    transposed output tiles then give 4 KB-contiguous DMA writes.
  * pass 2: +c is an ACT *bias* in the [d, token] layout (c is
    per-partition there) fused with relu; PE only does the 128x128
    transposes back; DVE drains PSUM into the bf16 output tile.
"""

import os
import sys

sys.path.insert(0, "/opt/trn_rl_repo")

import numpy as np
import ml_dtypes

N, D, K = 500000, 128, 32
CORES = 8
SUP = 2048                   # tokens per super-tile (one DMA)
JC = SUP // 128              # 16 rows per partition per super-tile
NSUP = 31                    # super-tiles per core
N_SH = NSUP * SUP            # 63488 padded rows per core
NT = NSUP * 4                # 124 compute tiles (512 tokens each)
N_PAD = CORES * N_SH         # 507904

_cache = {}
last_results = None


def _build():
    import concourse.bass as bass
    import concourse.bacc as bacc
    import concourse.tile as tile
    import concourse.mybir as mybir
    from concourse import masks

    f32 = mybir.dt.float32
    bf16 = mybir.dt.bfloat16
    AF = mybir.ActivationFunctionType
    ALU = mybir.AluOpType

    nc = bacc.Bacc("TRN2", target_bir_lowering=False, debug=False,
                   num_devices=CORES)

    x_d = nc.dram_tensor("x", [N_SH, D], bf16, kind="ExternalInput").ap()
    he_d = nc.dram_tensor("h_e", [N_SH, D], bf16, kind="ExternalInput").ap()
    bond_d = nc.dram_tensor("bond_n", [K, N_SH], bf16, kind="ExternalInput").ap()
    wi_d = nc.dram_tensor("W_i_w", [D, 2 * D], f32, kind="ExternalInput").ap()
    bi_d = nc.dram_tensor("W_i_b", [D], f32, kind="ExternalInput").ap()
    wm_d = nc.dram_tensor("W_m_w", [D, D], f32, kind="ExternalInput").ap()
    bm_d = nc.dram_tensor("W_m_b", [D], f32, kind="ExternalInput").ap()
    h_d = nc.dram_tensor("h", [N_SH, D], bf16, kind="ExternalOutput").ap()

    # output rows 2048*s + 16*p + j -> 4 KB contiguous per partition
    h_re = h_d.rearrange("(s p j) d -> s p j d", p=128, j=JC)
    bond_re = bond_d.rearrange("k (s u) -> s k u", u=SUP)

    with tile.TileContext(nc) as tc:
        import contextlib
        ctx = contextlib.ExitStack()
        with ctx:
            pers = ctx.enter_context(tc.tile_pool(name="pers", bufs=1))
            iosup = ctx.enter_context(tc.tile_pool(name="iosup", bufs=3))
            iosm = ctx.enter_context(tc.tile_pool(name="iosm", bufs=6))
            ioout = ctx.enter_context(tc.tile_pool(name="ioout", bufs=2))
            ps_t = ctx.enter_context(tc.tile_pool(name="ps_t", bufs=4, space="PSUM"))
            ps_z = ctx.enter_context(tc.tile_pool(name="ps_z", bufs=2, space="PSUM"))
            ps_w = ctx.enter_context(tc.tile_pool(name="ps_w", bufs=2, space="PSUM"))
            dram = ctx.enter_context(tc.tile_pool(name="dram", bufs=1, space="DRAM"))

            # ---- one-time setup -------------------------------------------
            ident_bf = pers.tile([128, 128], bf16)
            masks.make_identity(nc, ident_bf[:])
            ident_f = pers.tile([128, 128], f32)
            masks.make_identity(nc, ident_f[:])

            ones32 = pers.tile([K, 128], bf16)
            nc.gpsimd.memset(ones32[:], 1.0)

            b1_col = pers.tile([128, 1], f32)
            nc.sync.dma_start(b1_col[:, 0], bi_d[:])
            b2_col = pers.tile([128, 1], f32)
            nc.sync.dma_start(b2_col[:, 0], bm_d[:])

            # W_i_w [D, 2D] -> bf16 -> transpose halves -> W1aT/W1bT [j, d]
            wi_sb = pers.tile([128, 2 * D], f32)
            nc.sync.dma_start(wi_sb[:], wi_d[:])
            wi_bf = pers.tile([128, 2 * D], bf16)
            nc.vector.tensor_copy(wi_bf[:], wi_sb[:])
            w1t = pers.tile([128, 2 * D], bf16)  # [j, (half d)]
            for half in range(2):
                tp = ps_t.tile([128, 128], bf16, tag="tr")
                nc.tensor.transpose(tp[:], wi_bf[:, 128 * half:128 * (half + 1)],
                                    ident_bf[:])
                nc.vector.tensor_copy(w1t[:, 128 * half:128 * (half + 1)], tp[:])

            # W_m_w [D, D] -> WmT [d, d'] f32 (precision-critical path)
            wm_sb = pers.tile([128, D], f32)
            nc.sync.dma_start(wm_sb[:], wm_d[:])
            wmt = pers.tile([128, D], f32)
            tpm = ps_z.tile([128, 128], f32, tag="z")
            nc.tensor.transpose(tpm[:], wm_sb[:], ident_f[:])
            nc.vector.tensor_copy(wmt[:], tpm[:])

            m_parts = pers.tile([128, NT], f32)
            hn_all = pers.tile([128, N_SH], bf16)   # SBUF-resident h_n.T
            m_in = dram.tile([128], f32)
            m_out = dram.tile([128], f32, addr_space="Shared")

            # ---- pass 1 ----------------------------------------------------
            for s in range(NSUP):
                xt = iosup.tile([128, SUP], bf16)
                nc.sync.dma_start_transpose(
                    xt[:], x_d[SUP * s:SUP * (s + 1), :])
                het = iosup.tile([128, SUP], bf16)
                nc.sync.dma_start_transpose(
                    het[:], he_d[SUP * s:SUP * (s + 1), :])
                b_bf = iosup.tile([K, SUP], bf16)
                nc.gpsimd.dma_start(b_bf[:], bond_re[s])

                # matmuls all stream CONTIGUOUS columns (full PE rate);
                # the token permutation pi(u) = 128*(u%16) + u//16 is
                # applied by the ACT write below so that pass 2 reads
                # contiguous blocks of tokens {16p + j} (4 KB DMA writes).
                hn_sup = hn_all[:, SUP * s:SUP * (s + 1)]
                hn_perm = hn_sup.rearrange("d (b p) -> d b p", b=JC, p=128)

                for g in range(4):
                    z_ps = ps_z.tile([128, 512], f32, tag="z")
                    csl = slice(512 * g, 512 * (g + 1))
                    nc.tensor.matmul(z_ps[:], w1t[:, 0:128], xt[:, csl],
                                     start=True, stop=False)
                    nc.tensor.matmul(z_ps[:], w1t[:, 128:256], het[:, csl],
                                     start=False, stop=True)

                    wb_ps = ps_w.tile([128, 512], f32, tag="wb")
                    nc.tensor.matmul(wb_ps[:], ones32[:], b_bf[:, csl],
                                     start=True, stop=True)

                    ti = 4 * s + g
                    # dst: [d, b=16 (stride 128), p'=32 (stride 1)] at col
                    # offset 32g; src z column v=16a+b -> [d, b, a] view.
                    hn_v = hn_perm[:, :, 32 * g:32 * (g + 1)]
                    z_v = z_ps[:].rearrange("d (a b) -> d b a", a=32, b=JC)
                    nc.scalar.activation(hn_v, z_v, AF.Relu, bias=b1_col[:])
                    junk = iosm.tile([128, 512], bf16)
                    nc.vector.scalar_tensor_tensor(
                        junk[:].rearrange("d (a b) -> d b a", a=32, b=JC),
                        hn_v, 1.0,
                        wb_ps[:].rearrange("d (a b) -> d b a", a=32, b=JC),
                        ALU.mult, ALU.mult,
                        accum_out=m_parts[:, ti:ti + 1])

            # ---- m all-reduce + c ----------------------------------------
            m_col = pers.tile([128, 1], f32)
            nc.vector.reduce_sum(m_col[:], m_parts[:], axis=mybir.AxisListType.X)
            nc.sync.dma_start(m_in[:], m_col[:, 0])
            nc.gpsimd.collective_compute(
                "AllReduce", ALU.add,
                replica_groups=[list(range(CORES))],
                ins=[m_in[:].opt()], outs=[m_out[:].opt()])
            m_sb = pers.tile([128, 1], f32)
            nc.sync.dma_start(m_sb[:, 0], m_out[:])

            c_ps = ps_z.tile([128, 1], f32, tag="z")
            nc.tensor.matmul(c_ps[:], wmt[:], m_sb[:], start=True, stop=True)
            c_col = pers.tile([128, 1], f32)
            nc.vector.tensor_tensor(c_col[:], c_ps[:], b2_col[:], ALU.add)

            # ---- pass 2 ----------------------------------------------------
            for s in range(NSUP):
                h_sup = ioout.tile([128, SUP], bf16)
                for g in range(4):
                    ti = 4 * s + g
                    t_bf = iosm.tile([128, 512], bf16)
                    nc.scalar.activation(t_bf[:],
                                         hn_all[:, 512 * ti:512 * (ti + 1)],
                                         AF.Relu, bias=c_col[:])
                    ht_ps = ps_t.tile([128, 512], f32, tag="tr")
                    for jj in range(4):
                        psl = slice(128 * jj, 128 * (jj + 1))
                        nc.tensor.matmul(ht_ps[:, psl], t_bf[:, psl],
                                         ident_bf[:], start=True, stop=True,
                                         skip_group_check=True)
                    nc.vector.tensor_copy(h_sup[:, 512 * g:512 * (g + 1)],
                                          ht_ps[:])
                nc.scalar.dma_start(h_re[s],
                                    h_sup[:].rearrange("p (j d) -> p j d",
                                                       j=JC))

    nc.compile()
    return nc


def _get_nc():
    if "nc" not in _cache:
        _cache["nc"] = _build()
    return _cache["nc"]


def _ensure_ntff_hook():
    """Register the axon NTFF profile hook if the image's antenv lacks it."""
    import types
    try:
        import antenv.axon_hooks  # noqa: F401
        return
    except ImportError:
        pass
    try:
        import antenv
        from trn_agent_boot.trn_boot import _ntff_profile_via_ctypes
        mod = types.ModuleType("antenv.axon_hooks")
        _h = {"hook": None}
        mod.set_axon_ntff_profile_hook = lambda h: _h.__setitem__("hook", h)
        mod.get_axon_ntff_profile_hook = lambda: _h["hook"]
        sys.modules["antenv.axon_hooks"] = mod
        antenv.axon_hooks = mod
        hook = _ntff_profile_via_ctypes("/opt/axon/libaxon_pjrt.so")
        if hook is not None:
            mod.set_axon_ntff_profile_hook(hook)
    except Exception:
        pass


def kernel(**inputs):
    global last_results
    from concourse.bass_utils import run_bass_kernel_spmd

    bf = ml_dtypes.bfloat16
    x = np.asarray(inputs["x"], dtype=np.float32).astype(bf)
    he = np.asarray(inputs["h_e"], dtype=np.float32).astype(bf)
    bond = np.asarray(inputs["bond_n"], dtype=np.float32).astype(bf)
    wi = np.ascontiguousarray(np.asarray(inputs["W_i_w"], dtype=np.float32))
    bi = np.ascontiguousarray(np.asarray(inputs["W_i_b"], dtype=np.float32))
    wm = np.ascontiguousarray(np.asarray(inputs["W_m_w"], dtype=np.float32))
    bm = np.ascontiguousarray(np.asarray(inputs["W_m_b"], dtype=np.float32))

    n = x.shape[0]
    pad = N_PAD - n
    xp = np.concatenate([x, np.zeros((pad, D), bf)], 0)
    hep = np.concatenate([he, np.zeros((pad, D), bf)], 0)
    bondp = np.concatenate([bond, np.zeros((K, pad), bf)], 1)

    in_maps = []
    for c in range(CORES):
        sl = slice(c * N_SH, (c + 1) * N_SH)
        in_maps.append({
            "x": np.ascontiguousarray(xp[sl]),
            "h_e": np.ascontiguousarray(hep[sl]),
            "bond_n": np.ascontiguousarray(bondp[:, sl]),
            "W_i_w": wi, "W_i_b": bi, "W_m_w": wm, "W_m_b": bm,
        })

    nc = _get_nc()
    trace = os.environ.get("BASS_KERNEL_TRACE", "0") == "1"
    if trace:
        _ensure_ntff_hook()
    res = run_bass_kernel_spmd(nc, in_maps, core_ids=list(range(CORES)),
                               trace=trace)
    last_results = res
    out = np.concatenate([np.asarray(r["h"]) for r in res.results], 0)[:n]
    return np.ascontiguousarray(out.astype(np.float32))
